# revision 35
# baseline (speedup 1.0000x reference)
"""Trainium2 Bass kernel: PointPillarScatter + dense self-attention block.

Math identities exploited (host side, exact):
  - The grid indices are unique per sample, so scatter->RoPE->gather equals
    applying RoPE per token at its grid position, and the final output grid
    is exactly zero away from the pillar columns (rot == 0 where grid == 0).
  - LN1 is folded on host; LN2's g2/be2 and mean-subtraction are folded into
    an augmented W1 (extra contraction row carrying the per-token mean).

Device kernel (SPMD over 8 cores = 2 samples x 4 query shards):
  QKV projections, S^T = K @ Q^T scores (keys on partitions), exp on ScalarE,
  PV with an extra all-ones V column producing the softmax denominators inside
  the same PSUM accumulation, the un-normalize + residual, LN2 stats via
  ones-matmuls, and the GELU FFN. All matmuls bf16 with fp32 PSUM; residual
  paths fp32.
"""

import os
import sys

import numpy as np

sys.path.insert(0, "/opt/trn_rl_repo")

import concourse.bass as bass  # noqa: E402
import concourse.bacc as bacc  # noqa: E402
import concourse.tile as tile  # noqa: E402
from concourse import mybir  # noqa: E402

dt = mybir.dt
F32 = dt.float32
BF16 = dt.bfloat16
FP8 = dt.float8e4
AF = mybir.ActivationFunctionType
ALU = mybir.AluOpType
NP_BF16 = dt.np(BF16)

B, P, C, HALF = 2, 6000, 64, 32
NX, NY = 432, 496
HW = NX * NY
NCORES = 8
QSH = 4             # query shards per sample
PQR = P // QSH      # 1500 real queries per core
QC = 512            # query chunk (one PSUM bank of fp32)
NCH = 3             # chunks per core
PQ = QC * NCH       # 1536 padded queries per core
NKT = 47            # key tiles of 128
KP = NKT * 128      # 6016 padded keys
VS = 66             # PSUM stride of one V tile (65 cols used, 8B aligned)
VS8 = 80            # SBUF stride of one V tile in the PV weight buffer (16B aligned)
EXPG = 3            # PSUM banks (key tiles) per exp group
PACK = int(os.environ.get("KPACK", "1"))  # row-packed paired S^T matmuls
KF8 = int(os.environ.get("KF8", "1"))     # fp8 DoubleRow PV matmuls
KWARM = int(os.environ.get("KWARM", "16"))  # PE warmup matmuls


def tok_of_slot(s):
    """Consumption slot -> token tile. With PACK, even slots are the A half
    (token tiles 0..23, kt2 partitions 0:64), odd the B half (24..46)."""
    if not PACK:
        return s
    return (s // 2) if s % 2 == 0 else 24 + (s // 2)


def _build_program():
    nc = bacc.Bacc("TRN2", target_bir_lowering=False, debug=False)

    def prm(name, shape, d, out=False):
        return nc.declare_dram_parameter(name, list(shape), d, isOutput=out)

    n1t = prm("n1t", (128, KP), BF16)       # LN1'd feats^T, duplicated in both halves
    qn1t = prm("qn1t", (128, PQ), BF16)     # query-side feats^T, duplicated halves
    rawqt = prm("rawqt", (64, PQ), F32)     # raw feats^T (residual) for queries
    wq = prm("wq", (128, 64), BF16)         # weights duplicated in both halves
    bq = prm("bq", (64, 1), F32)
    wk = prm("wk", (128, 64), BF16)
    bk = prm("bk", (64, 1), F32)
    wv = prm("wv", (128, 64), BF16)
    bv = prm("bv", (64, 1), F32)            # folded in after softmax (sum w = 1)
    w1aug = prm("w1aug", (65, 64), BF16)    # [g2*W1 ; -colsum(g2*W1)]
    b1f = prm("b1f", (64, 1), F32)          # be2 @ W1 + b1
    w2 = prm("w2", (64, 64), BF16)
    b2 = prm("b2", (64, 1), F32)
    updt = prm("updt", (64, PQ), F32, True)  # output: updated feats^T for queries

    groups = [(k0, min(EXPG, NKT - k0)) for k0 in range(0, NKT, EXPG)]

    with tile.TileContext(nc) as tc:
        with tc.tile_pool(name="const", bufs=1) as const, \
             tc.tile_pool(name="big", bufs=1) as big, \
             tc.tile_pool(name="work", bufs=3) as work, \
             tc.tile_pool(name="om", bufs=2) as om:

            # ---------------- persistent SBUF ----------------
            sb_n1t = big.tile([128, KP], BF16)
            nc.sync.dma_start(sb_n1t[:], n1t[:])
            sb_qn1t = big.tile([128, PQ], BF16)
            nc.sync.dma_start(sb_qn1t[:], qn1t[:])
            sb_rawqt = big.tile([64, PQ], F32)
            nc.sync.dma_start(sb_rawqt[:], rawqt[:])

            def ld(p, shape, d, name):
                t = const.tile(list(shape), d, tag=name)
                nc.sync.dma_start(t[:], p[:])
                return t

            sb_wq = ld(wq, (128, 64), BF16, "wq")
            sb_bq = ld(bq, (64, 1), F32, "bq")
            sb_wk = ld(wk, (128, 64), BF16, "wk")
            sb_bk = ld(bk, (64, 1), F32, "bk")
            sb_wv = ld(wv, (128, 64), BF16, "wv")
            sb_bv = ld(bv, (64, 1), F32, "bv")
            sb_w1aug = ld(w1aug, (65, 64), BF16, "w1aug")
            sb_b1f = ld(b1f, (64, 1), F32, "b1f")
            sb_w2 = ld(w2, (64, 64), BF16, "w2")
            sb_b2 = ld(b2, (64, 1), F32, "b2")

            sb_zero = const.tile([128, 1], F32)
            nc.vector.memset(sb_zero[:], 0.0)
            nc.const_aps.aps[(F32, 0.0)] = sb_zero[:]
            sb_eps = const.tile([128, 1], F32)
            nc.vector.memset(sb_eps[:], 1e-5)
            nc.const_aps.aps[(F32, 1e-5)] = sb_eps[:]

            sb_o128 = const.tile([1, 128], F32)
            nc.vector.memset(sb_o128[:], 1.0)
            sb_o64b = const.tile([1, 64], BF16)
            nc.vector.memset(sb_o64b[:], 1.0)
            sb_oc64 = const.tile([64, 1], BF16)
            nc.vector.memset(sb_oc64[:], 1.0 / 64.0)

            # Q^T duplicated into both partition halves (rhs for row-packed S^T)
            sb_qtd = big.tile([128 if PACK else 64, PQ], BF16)
            # K^T split-half layout: [0:64] = token tiles 0..23, [64:128] = 24..46
            sb_kt2 = big.tile([128, 24 * 128] if PACK else [64, KP], BF16)
            sb_v8 = big.tile([128, 48, VS8], FP8 if KF8 else BF16)
            sb_att = big.tile([64, PQ], F32)     # att = attn@V/den + raw
            sb_attB = big.tile([65, PQ], BF16)   # bf16 att + mean row
            sb_vr = big.tile([1, PQ], F32)       # LN2 variance row
            sb_dinv = big.tile([1, PQ], F32)     # softmax 1/denominator
            sb_rs = big.tile([1, PQ], F32)       # LN2 1/sigma
            sb_rsb = big.tile([1, PQ], BF16)

            # ---------------- Phase A: Q, K, V ----------------
            with tc.tile_pool(name="psA", bufs=3, space="PSUM") as psA:
                if KWARM:
                    # spin the PE while input DMAs stream so HAM un-throttles
                    # (4096-cycle busy window) before the real matmuls start
                    sb_wu = const.tile([64, 512], BF16, tag="warm")
                    nc.vector.memset(sb_wu[:], 0.0)
                    psW = psA.tile([128, QC], F32, tag="qk", bufs=4)
                    for _ in range(KWARM):
                        nc.tensor.matmul(psW[:], sb_wu[:, 0:128], sb_wu[:],
                                         start=True, stop=True)
                state_a = []

                def proj_pair(jobs):
                    # jobs: list of (weights, half_lo, src_cols_n, dst, bias)
                    pss = []
                    for (w, lo, cols, n, src, dst, bias) in jobs:
                        ps = psA.tile([64, QC], F32, tag="qk", bufs=4,
                                      name=f"psQK_{len(state_a)}")
                        state_a.append(0)
                        nc.tensor.matmul(ps[:, :n], w[lo:lo + 64, :],
                                         src[lo:lo + 64, cols:cols + n],
                                         start=True, stop=True,
                                         tile_position=(lo, 0) if PACK else None)
                        pss.append((ps, n, dst, bias))
                    for (ps, n, dst, bias) in pss:
                        nc.vector.tensor_scalar_add(dst, ps[:, :n], bias)

                # Q: chunks 0..2 (pair first two across halves when packing)
                if PACK:
                    qjobs = [[(sb_wq, 0, 0, QC, sb_qn1t, None, sb_bq),
                              (sb_wq, 64, QC, QC, sb_qn1t, None, sb_bq)],
                             [(sb_wq, 0, 2 * QC, QC, sb_qn1t, None, sb_bq)]]
                else:
                    qjobs = [[(sb_wq, 0, j * QC, QC, sb_qn1t, None, sb_bq)]
                             for j in range(NCH)]
                for jobs in qjobs:
                    jj = [(w, lo, cols, n, src,
                           sb_qtd[0:64, cols:cols + n], bias)
                          for (w, lo, cols, n, src, _, bias) in jobs]
                    proj_pair(jj)
                if PACK:
                    for j in range(NCH):
                        nc.vector.tensor_copy(sb_qtd[64:128, j * QC:(j + 1) * QC],
                                              sb_qtd[0:64, j * QC:(j + 1) * QC])

                # K: 12 chunks; pair (j, j+6) across halves when packing
                if PACK:
                    for j0 in range(6):
                        nB = min(QC, KP - (j0 + 6) * QC)
                        jobs = [(sb_wk, 0, j0 * QC, QC, sb_n1t,
                                 sb_kt2[0:64, j0 * QC:(j0 + 1) * QC], sb_bk),
                                (sb_wk, 64, (j0 + 6) * QC, nB, sb_n1t,
                                 sb_kt2[64:128, j0 * QC:j0 * QC + nB], sb_bk)]
                        proj_pair(jobs)
                else:
                    nkc = (KP + QC - 1) // QC
                    for j in range(nkc):
                        n = min(QC, KP - j * QC)
                        jobs = [(sb_wk, 0, j * QC, n, sb_n1t,
                                 sb_kt2[0:64, j * QC:j * QC + n], sb_bk)]
                        proj_pair(jobs)
                # softmax-denominator ones column; pad keys (rows 112:128 of
                # the pad slot) zeroed via legal 32-aligned partition bases
                pad_slot = 2 * (46 - 24) + 1 if PACK else 46
                nc.vector.memset(sb_v8[:, :, 64:65], 1.0)
                nc.vector.memset(sb_v8[96:128, pad_slot, 64:65], 0.0)
                nc.vector.memset(sb_v8[96:112, pad_slot, 64:65], 1.0)

            # ---------------- Phase B: attention ----------------
            with tc.tile_pool(name="psS", bufs=2, space="PSUM") as psS_pool, \
                 tc.tile_pool(name="psP", bufs=1, space="PSUM") as psP_pool, \
                 tc.tile_pool(name="psB", bufs=1, space="PSUM") as psB_pool:
                vstate = {"done": 0}

                def emit_vbatch():
                    # V projections ride the psS-tag PSUM rotation during
                    # chunk 0 so the first exps are not blocked behind them.
                    sv0 = vstate["done"]
                    if sv0 >= NKT:
                        return
                    svn = min(6, NKT - sv0)
                    vt = psS_pool.tile([128, EXPG, QC], F32, tag="s",
                                       name=f"psV_{sv0}")
                    for t in range(svn):
                        s = sv0 + t
                        tok = tok_of_slot(s)
                        lo = 64 * (s % 2) if PACK else 0
                        h, i = t % 2, t // 2
                        nc.tensor.matmul(vt[:, h, i * VS:i * VS + 64],
                                         sb_n1t[lo:lo + 64, tok * 128:(tok + 1) * 128],
                                         sb_wv[lo:lo + 64, :], start=True, stop=True,
                                         tile_position=(lo, 0) if PACK else None)
                    ne, no = (svn + 1) // 2, svn // 2
                    ve = vt[:, 0, 0:ne * VS].rearrange("p (i v) -> p i v", v=VS)
                    nc.vector.tensor_copy(sb_v8[:, sv0:sv0 + svn:2, 0:64],
                                          ve[:, :, 0:64])
                    if no:
                        vo = vt[:, 1, 0:no * VS].rearrange("p (i v) -> p i v", v=VS)
                        nc.vector.tensor_copy(sb_v8[:, sv0 + 1:sv0 + svn:2, 0:64],
                                              vo[:, :, 0:64])
                    vstate["done"] += svn

                for j in range(NCH):
                    qs = slice(j * QC, (j + 1) * QC)
                    sb_exp = big.tile([128, NKT, QC], FP8 if KF8 else BF16, tag="expS")
                    psP = psP_pool.tile([65, QC], F32, tag="pv")

                    state = {"pv": 0, "exp": 0}

                    def pv_upto(limit):
                        while True:
                            s = state["pv"]
                            if s >= NKT:
                                break
                            if KF8 and s % 2 == 0 and s + 1 < NKT:
                                if s + 2 > limit:
                                    break
                                nc.tensor.matmul(psP[:], sb_v8[:, s:s + 2, 0:65],
                                                 sb_exp[:, s:s + 2, :],
                                                 start=(s == 0), stop=False,
                                                 perf_mode=mybir.MatmulPerfMode.DoubleRow)
                                state["pv"] += 2
                            else:
                                if s + 1 > limit:
                                    break
                                nc.tensor.matmul(psP[:], sb_v8[:, s, 0:65],
                                                 sb_exp[:, s, :],
                                                 start=(s == 0), stop=(s == NKT - 1))
                                state["pv"] += 1

                    gtiles = {}
                    filled = [0] * len(groups)
                    if PACK:
                        steps = [[2 * p, 2 * p + 1] if p < 23 else [46] for p in range(24)]
                    else:
                        steps = [[s] for s in range(NKT)]
                    for slots in steps:
                        for s in slots:
                            g = s // 3
                            if g not in gtiles:
                                gtiles[g] = psS_pool.tile([128, EXPG * QC], F32, tag="s",
                                                          name=f"psS_{j}_{g}")
                        for hi, s in enumerate(slots):
                            lo = 64 * hi
                            tok = tok_of_slot(s) if PACK else s
                            col = (tok % 24) if PACK else tok
                            nc.tensor.matmul(
                                gtiles[s // 3][:, (s % 3) * QC:(s % 3 + 1) * QC],
                                sb_kt2[lo:lo + 64, col * 128:(col + 1) * 128],
                                sb_qtd[lo:lo + 64, qs], start=True, stop=True,
                                tile_position=(lo, 0) if PACK else None)
                        for s in slots:
                            g = s // 3
                            filled[g] += 1
                            if filled[g] == groups[g][1]:
                                gn = groups[g][1]
                                nc.scalar.activation(
                                    sb_exp[:, 3 * g:3 * g + gn, :],
                                    gtiles[g][:, :gn * QC], AF.Exp)
                                del gtiles[g]
                                state["exp"] += 1
                                emit_vbatch()
                                if state["exp"] >= 2:
                                    pv_upto(min(3 * (state["exp"] - 1),
                                                vstate["done"]))
                    pv_upto(NKT)

                    # softmax denominator + residual
                    if int(os.environ.get("KFASTRECIP", "0")):
                        nc.vector.reciprocal_approx_fast(sb_dinv[:, qs], psP[64:65, :])
                    else:
                        nc.vector.reciprocal(sb_dinv[:, qs], psP[64:65, :])
                    psR = psB_pool.tile([128, QC], F32, tag="bc")
                    nc.tensor.matmul(psR[:], sb_o128[:], sb_dinv[:, qs], start=True, stop=True)
                    sb_rb = work.tile([64, QC], F32, tag="rb")
                    nc.vector.tensor_copy(sb_rb[:], psR[0:64, :])
                    sb_a0 = work.tile([64, QC], F32, tag="a0")
                    nc.vector.tensor_mul(sb_a0[:], psP[0:64, :], sb_rb[:])
                    nc.vector.scalar_tensor_tensor(sb_att[:, qs], sb_a0[:], sb_bv[:],
                                                   sb_rawqt[:, qs],
                                                   op0=ALU.add, op1=ALU.add)
                    nc.vector.tensor_copy(sb_attB[0:64, qs], sb_att[:, qs])

                    # LN2 stats: mean row into attB[64], variance row into sb_vr
                    psM = psB_pool.tile([128, QC], F32, tag="bc")
                    nc.tensor.matmul(psM[0:1, :], sb_oc64[:], sb_attB[0:64, qs],
                                     start=True, stop=True)
                    nc.vector.tensor_copy(sb_attB[64:65, qs], psM[0:1, :])
                    sb_sq = work.tile([64, QC], BF16, tag="sq")
                    nc.vector.tensor_mul(sb_sq[:], sb_att[:, qs], sb_att[:, qs])
                    psM2 = psB_pool.tile([128, QC], F32, tag="bc")
                    nc.tensor.matmul(psM2[0:1, :], sb_oc64[:], sb_sq[:], start=True, stop=True)
                    sb_m2 = work.tile([1, QC], F32, tag="m2")
                    nc.vector.tensor_mul(sb_m2[:], sb_attB[64:65, qs], sb_attB[64:65, qs])
                    nc.vector.tensor_sub(sb_vr[:, qs], psM2[0:1, :], sb_m2[:])

            # ---------------- Phase C: FFN ----------------
            with tc.tile_pool(name="psC", bufs=2, space="PSUM") as psC:
                sb_lv = big.tile([1, PQ], F32)
                nc.scalar.activation(sb_lv[:], sb_vr[:], AF.Ln, bias=1e-5)
                nc.scalar.activation(sb_rs[:], sb_lv[:], AF.Exp, scale=-0.5)
                nc.vector.tensor_copy(sb_rsb[:], sb_rs[:])
                for j in range(NCH):
                    qs = slice(j * QC, (j + 1) * QC)
                    psR2 = psC.tile([64, QC], F32, tag="rs")
                    nc.tensor.matmul(psR2[:], sb_o64b[:], sb_rsb[:, qs], start=True, stop=True)
                    sb_rsB = work.tile([64, QC], F32, tag="rsB")
                    nc.vector.tensor_copy(sb_rsB[:], psR2[:])
                    psH = psC.tile([64, QC], F32, tag="h")
                    nc.tensor.matmul(psH[:], sb_w1aug[:], sb_attB[:, qs], start=True, stop=True)
                    sb_hp = work.tile([64, QC], F32, tag="hp")
                    nc.vector.tensor_mul(sb_hp[:], psH[:], sb_rsB[:])
                    sb_gh = work.tile([64, QC], BF16, tag="gh")
                    nc.scalar.activation(sb_gh[:], sb_hp[:], AF.Gelu, bias=sb_b1f[:])
                    psU = psC.tile([64, QC], F32, tag="u")
                    nc.tensor.matmul(psU[:], sb_w2[:], sb_gh[:], start=True, stop=True)
                    sb_out = om.tile([64, QC], F32, tag="o")
                    nc.vector.scalar_tensor_tensor(sb_out[:], psU[:], sb_b2[:],
                                                   sb_att[:, qs], op0=ALU.add, op1=ALU.add)
                    nc.sync.dma_start(updt[:, qs], sb_out[:])
    nc.compile()
    return nc


_CACHE = {}


def _get_program():
    if "nc" not in _CACHE:
        _CACHE["nc"] = _build_program()
    return _CACHE["nc"]


def _host_prep(pillar_features, voxel_coords, Wq, bq, Wk, bk, Wv, bv,
               W1, b1, W2, b2, g1, be1, g2, be2):
    """Returns (in_maps, idx). All heavy parameterized compute stays on device."""
    pf = np.asarray(pillar_features, np.float32).reshape(B, P, C)
    vc = np.asarray(voxel_coords).astype(np.int64)
    idx = (vc[:, 1] + vc[:, 2] * NX + vc[:, 3]).reshape(B, P)

    theta = 10000.0 ** (-np.arange(HALF, dtype=np.float64) / HALF)
    h = np.linspace(-1.0, 1.0, NY)
    w = np.linspace(-1.0, 1.0, NX)
    hrow = h[idx // NX]
    wcol = w[idx % NX]
    A = np.cos(hrow[..., None] * theta) * np.cos(wcol[..., None] * theta)
    Bt = np.sin(hrow[..., None] * theta) * np.sin(wcol[..., None] * theta)
    x1 = pf[..., :HALF].astype(np.float64)
    x2 = pf[..., HALF:].astype(np.float64)
    raw = np.concatenate([x1 + x1 * A - x2 * Bt, x2 + x1 * Bt + x2 * A], axis=-1)

    g1 = np.asarray(g1, np.float64)
    be1 = np.asarray(be1, np.float64)
    m = raw.mean(-1, keepdims=True)
    v = ((raw - m) ** 2).mean(-1, keepdims=True)
    n1 = (raw - m) / np.sqrt(v + 1e-5) * g1 + be1

    raw32 = raw.astype(np.float32)
    n1b = n1.astype(NP_BF16)

    W1 = np.asarray(W1, np.float32)
    g2 = np.asarray(g2, np.float32)
    be2 = np.asarray(be2, np.float32)
    W1g = g2[:, None] * W1
    w1aug = np.zeros((65, 64), np.float32)
    w1aug[:64] = W1g
    w1aug[64] = -W1g.sum(axis=0)
    b1f = (be2 @ W1 + np.asarray(b1, np.float32)).reshape(64, 1)

    def dup(a):
        return np.vstack([a, a])

    weights = dict(
        wq=dup(np.asarray(Wq, np.float32).astype(NP_BF16)),
        bq=np.asarray(bq, np.float32).reshape(64, 1),
        wk=dup(np.asarray(Wk, np.float32).astype(NP_BF16)),
        bk=np.asarray(bk, np.float32).reshape(64, 1),
        wv=dup(np.asarray(Wv, np.float32).astype(NP_BF16)),
        bv=np.asarray(bv, np.float32).reshape(64, 1),
        w1aug=w1aug.astype(NP_BF16),
        b1f=b1f.astype(np.float32),
        w2=np.asarray(W2, np.float32).astype(NP_BF16),
        b2=np.asarray(b2, np.float32).reshape(64, 1).astype(np.float32),
    )

    n1t_s = []
    for b in range(B):
        t = np.zeros((64, KP), NP_BF16)
        t[:, :P] = n1b[b].T
        n1t_s.append(dup(t))

    in_maps = []
    for core in range(NCORES):
        b, s = divmod(core, QSH)
        q0 = s * PQR
        qn1t = np.zeros((64, PQ), NP_BF16)
        qn1t[:, :PQR] = n1b[b, q0:q0 + PQR].T
        rawqt = np.zeros((64, PQ), np.float32)
        rawqt[:, :PQR] = raw32[b, q0:q0 + PQR].T
        in_maps.append(dict(n1t=n1t_s[b], qn1t=dup(qn1t), rawqt=rawqt, **weights))
    return in_maps, idx


def _assemble(results, idx):
    out = np.zeros((B, C, HW), np.float32)
    for core in range(NCORES):
        b, s = divmod(core, QSH)
        q0 = s * PQR
        updt = np.asarray(results[core]["updt"], np.float32)
        out[b][:, idx[b, q0:q0 + PQR]] = updt[:, :PQR]
    return out.reshape(B, C, NY, NX)


def _run_device(in_maps, **kwargs):
    from concourse.bass_utils import run_bass_kernel_spmd
    nc = _get_program()
    return run_bass_kernel_spmd(nc, in_maps, list(range(NCORES)), **kwargs)


def kernel(**inputs):
    in_maps, idx = _host_prep(**inputs)
    res = _run_device(in_maps)
    return _assemble(res.results, idx)


if __name__ == "__main__":
    nc = _get_program()
    print("program built OK")


# revision 36
# speedup vs baseline: 1.0128x; 1.0128x over previous
"""Trainium2 Bass kernel: PointPillarScatter + dense self-attention block.

Math identities exploited (host side, exact):
  - The grid indices are unique per sample, so scatter->RoPE->gather equals
    applying RoPE per token at its grid position, and the final output grid
    is exactly zero away from the pillar columns (rot == 0 where grid == 0).
  - LN1 is folded on host; LN2's g2/be2 and mean-subtraction are folded into
    an augmented W1 (extra contraction row carrying the per-token mean).

Device kernel (SPMD over 8 cores = 2 samples x 4 query shards):
  QKV projections, S^T = K @ Q^T scores (keys on partitions), exp on ScalarE,
  PV with an extra all-ones V column producing the softmax denominators inside
  the same PSUM accumulation, the un-normalize + residual, LN2 stats via
  ones-matmuls, and the GELU FFN. All matmuls bf16 with fp32 PSUM; residual
  paths fp32.
"""

import os
import sys

import numpy as np

sys.path.insert(0, "/opt/trn_rl_repo")

import concourse.bass as bass  # noqa: E402
import concourse.bacc as bacc  # noqa: E402
import concourse.tile as tile  # noqa: E402
from concourse import mybir  # noqa: E402

dt = mybir.dt
F32 = dt.float32
BF16 = dt.bfloat16
FP8 = dt.float8e4
AF = mybir.ActivationFunctionType
ALU = mybir.AluOpType
NP_BF16 = dt.np(BF16)

B, P, C, HALF = 2, 6000, 64, 32
NX, NY = 432, 496
HW = NX * NY
NCORES = 8
QSH = 4             # query shards per sample
PQR = P // QSH      # 1500 real queries per core
QC = 512            # query chunk (one PSUM bank of fp32)
NCH = 3             # chunks per core
PQ = QC * NCH       # 1536 padded queries per core
NKT = 47            # key tiles of 128
KP = NKT * 128      # 6016 padded keys
VS = 66             # PSUM stride of one V tile (65 cols used, 8B aligned)
VS8 = 80            # SBUF stride of one V tile in the PV weight buffer (16B aligned)
EXPG = 3            # PSUM banks (key tiles) per exp group
PACK = int(os.environ.get("KPACK", "1"))  # row-packed paired S^T matmuls
KF8 = int(os.environ.get("KF8", "1"))     # fp8 DoubleRow PV matmuls
KWARM = int(os.environ.get("KWARM", "0"))  # PE warmup matmuls


def tok_of_slot(s):
    """Consumption slot -> token tile. With PACK, even slots are the A half
    (token tiles 0..23, kt2 partitions 0:64), odd the B half (24..46)."""
    if not PACK:
        return s
    return (s // 2) if s % 2 == 0 else 24 + (s // 2)


def _build_program():
    nc = bacc.Bacc("TRN2", target_bir_lowering=False, debug=False)

    def prm(name, shape, d, out=False):
        return nc.declare_dram_parameter(name, list(shape), d, isOutput=out)

    n1t = prm("n1t", (128, KP), BF16)       # LN1'd feats^T, duplicated in both halves
    qn1t = prm("qn1t", (128, PQ), BF16)     # query-side feats^T, duplicated halves
    rawqt = prm("rawqt", (64, PQ), F32)     # raw feats^T (residual) for queries
    wq = prm("wq", (128, 64), BF16)         # weights duplicated in both halves
    bq = prm("bq", (64, 1), F32)
    wk = prm("wk", (128, 64), BF16)
    bk = prm("bk", (64, 1), F32)
    wv = prm("wv", (128, 64), BF16)
    bv = prm("bv", (64, 1), F32)            # folded in after softmax (sum w = 1)
    w1aug = prm("w1aug", (65, 64), BF16)    # [g2*W1 ; -colsum(g2*W1)]
    b1f = prm("b1f", (64, 1), F32)          # be2 @ W1 + b1
    w2 = prm("w2", (64, 64), BF16)
    b2 = prm("b2", (64, 1), F32)
    updt = prm("updt", (64, PQ), F32, True)  # output: updated feats^T for queries

    groups = [(k0, min(EXPG, NKT - k0)) for k0 in range(0, NKT, EXPG)]

    with tile.TileContext(nc) as tc:
        with tc.tile_pool(name="const", bufs=1) as const, \
             tc.tile_pool(name="big", bufs=1) as big, \
             tc.tile_pool(name="work", bufs=3) as work, \
             tc.tile_pool(name="om", bufs=2) as om:

            # ---------------- persistent SBUF ----------------
            sb_n1t = big.tile([128, KP], BF16)
            for c0 in range(0, KP, QC):
                cn = min(QC, KP - c0)
                nc.sync.dma_start(sb_n1t[:, c0:c0 + cn], n1t[:, c0:c0 + cn])
            sb_qn1t = big.tile([128, PQ], BF16)
            nc.sync.dma_start(sb_qn1t[:], qn1t[:])
            sb_rawqt = big.tile([64, PQ], F32)
            nc.sync.dma_start(sb_rawqt[:], rawqt[:])

            def ld(p, shape, d, name):
                t = const.tile(list(shape), d, tag=name)
                nc.sync.dma_start(t[:], p[:])
                return t

            sb_wq = ld(wq, (128, 64), BF16, "wq")
            sb_bq = ld(bq, (64, 1), F32, "bq")
            sb_wk = ld(wk, (128, 64), BF16, "wk")
            sb_bk = ld(bk, (64, 1), F32, "bk")
            sb_wv = ld(wv, (128, 64), BF16, "wv")
            sb_bv = ld(bv, (64, 1), F32, "bv")
            sb_w1aug = ld(w1aug, (65, 64), BF16, "w1aug")
            sb_b1f = ld(b1f, (64, 1), F32, "b1f")
            sb_w2 = ld(w2, (64, 64), BF16, "w2")
            sb_b2 = ld(b2, (64, 1), F32, "b2")

            sb_zero = const.tile([128, 1], F32)
            nc.vector.memset(sb_zero[:], 0.0)
            nc.const_aps.aps[(F32, 0.0)] = sb_zero[:]
            sb_eps = const.tile([128, 1], F32)
            nc.vector.memset(sb_eps[:], 1e-5)
            nc.const_aps.aps[(F32, 1e-5)] = sb_eps[:]

            sb_o128 = const.tile([1, 128], F32)
            nc.vector.memset(sb_o128[:], 1.0)
            sb_o64b = const.tile([1, 64], BF16)
            nc.vector.memset(sb_o64b[:], 1.0)
            sb_oc64 = const.tile([64, 1], BF16)
            nc.vector.memset(sb_oc64[:], 1.0 / 64.0)

            # Q^T duplicated into both partition halves (rhs for row-packed S^T)
            sb_qtd = big.tile([128 if PACK else 64, PQ], BF16)
            # K^T split-half layout: [0:64] = token tiles 0..23, [64:128] = 24..46
            sb_kt2 = big.tile([128, 24 * 128] if PACK else [64, KP], BF16)
            sb_v8 = big.tile([128, 48, VS8], FP8 if KF8 else BF16)
            sb_att = big.tile([64, PQ], F32)     # att = attn@V/den + raw
            sb_attB = big.tile([65, PQ], BF16)   # bf16 att + mean row
            sb_vr = big.tile([1, PQ], F32)       # LN2 variance row
            sb_dinv = big.tile([1, PQ], F32)     # softmax 1/denominator
            sb_rs = big.tile([1, PQ], F32)       # LN2 1/sigma
            sb_rsb = big.tile([1, PQ], BF16)

            # ---------------- Phase A: Q, K, V ----------------
            with tc.tile_pool(name="psA", bufs=3, space="PSUM") as psA:
                if KWARM:
                    # spin the PE while input DMAs stream so HAM un-throttles
                    # (4096-cycle busy window) before the real matmuls start
                    sb_wu = const.tile([64, 512], BF16, tag="warm")
                    nc.vector.memset(sb_wu[:], 0.0)
                    psW = psA.tile([128, QC], F32, tag="qk", bufs=4)
                    for _ in range(KWARM):
                        nc.tensor.matmul(psW[:], sb_wu[:, 0:128], sb_wu[:],
                                         start=True, stop=True)
                state_a = []

                def proj_pair(jobs):
                    # jobs: list of (weights, half_lo, src_cols_n, dst, bias)
                    pss = []
                    for (w, lo, cols, n, src, dst, bias) in jobs:
                        ps = psA.tile([64, QC], F32, tag="qk", bufs=4,
                                      name=f"psQK_{len(state_a)}")
                        state_a.append(0)
                        nc.tensor.matmul(ps[:, :n], w[lo:lo + 64, :],
                                         src[lo:lo + 64, cols:cols + n],
                                         start=True, stop=True,
                                         tile_position=(lo, 0) if PACK else None)
                        pss.append((ps, n, dst, bias))
                    for (ps, n, dst, bias) in pss:
                        nc.vector.tensor_scalar_add(dst, ps[:, :n], bias)

                # Q: chunks 0..2 (pair first two across halves when packing)
                if PACK:
                    qjobs = [[(sb_wq, 0, 0, QC, sb_qn1t, None, sb_bq),
                              (sb_wq, 64, QC, QC, sb_qn1t, None, sb_bq)],
                             [(sb_wq, 0, 2 * QC, QC, sb_qn1t, None, sb_bq)]]
                else:
                    qjobs = [[(sb_wq, 0, j * QC, QC, sb_qn1t, None, sb_bq)]
                             for j in range(NCH)]
                for jobs in qjobs:
                    jj = [(w, lo, cols, n, src,
                           sb_qtd[0:64, cols:cols + n], bias)
                          for (w, lo, cols, n, src, _, bias) in jobs]
                    proj_pair(jj)
                if PACK:
                    for j in range(NCH):
                        nc.vector.tensor_copy(sb_qtd[64:128, j * QC:(j + 1) * QC],
                                              sb_qtd[0:64, j * QC:(j + 1) * QC])

                # K: 12 chunks; pair (j, j+6) across halves when packing
                if PACK:
                    for j0 in range(6):
                        nB = min(QC, KP - (j0 + 6) * QC)
                        jobs = [(sb_wk, 0, j0 * QC, QC, sb_n1t,
                                 sb_kt2[0:64, j0 * QC:(j0 + 1) * QC], sb_bk),
                                (sb_wk, 64, (j0 + 6) * QC, nB, sb_n1t,
                                 sb_kt2[64:128, j0 * QC:j0 * QC + nB], sb_bk)]
                        proj_pair(jobs)
                else:
                    nkc = (KP + QC - 1) // QC
                    for j in range(nkc):
                        n = min(QC, KP - j * QC)
                        jobs = [(sb_wk, 0, j * QC, n, sb_n1t,
                                 sb_kt2[0:64, j * QC:j * QC + n], sb_bk)]
                        proj_pair(jobs)
                # softmax-denominator ones column; pad keys (rows 112:128 of
                # the pad slot) zeroed via legal 32-aligned partition bases
                pad_slot = 2 * (46 - 24) + 1 if PACK else 46
                nc.vector.memset(sb_v8[:, :, 64:65], 1.0)
                nc.vector.memset(sb_v8[96:128, pad_slot, 64:65], 0.0)
                nc.vector.memset(sb_v8[96:112, pad_slot, 64:65], 1.0)

            # ---------------- Phase B: attention ----------------
            with tc.tile_pool(name="psS", bufs=2, space="PSUM") as psS_pool, \
                 tc.tile_pool(name="psP", bufs=1, space="PSUM") as psP_pool, \
                 tc.tile_pool(name="psB", bufs=1, space="PSUM") as psB_pool:
                vstate = {"done": 0}

                def emit_vbatch():
                    # V projections ride the psS-tag PSUM rotation during
                    # chunk 0 so the first exps are not blocked behind them.
                    sv0 = vstate["done"]
                    if sv0 >= NKT:
                        return
                    svn = min(6, NKT - sv0)
                    vt = psS_pool.tile([128, EXPG, QC], F32, tag="s",
                                       name=f"psV_{sv0}")
                    for t in range(svn):
                        s = sv0 + t
                        tok = tok_of_slot(s)
                        lo = 64 * (s % 2) if PACK else 0
                        h, i = t % 2, t // 2
                        nc.tensor.matmul(vt[:, h, i * VS:i * VS + 64],
                                         sb_n1t[lo:lo + 64, tok * 128:(tok + 1) * 128],
                                         sb_wv[lo:lo + 64, :], start=True, stop=True,
                                         tile_position=(lo, 0) if PACK else None)
                    ne, no = (svn + 1) // 2, svn // 2
                    ve = vt[:, 0, 0:ne * VS].rearrange("p (i v) -> p i v", v=VS)
                    nc.vector.tensor_copy(sb_v8[:, sv0:sv0 + svn:2, 0:64],
                                          ve[:, :, 0:64])
                    if no:
                        vo = vt[:, 1, 0:no * VS].rearrange("p (i v) -> p i v", v=VS)
                        nc.vector.tensor_copy(sb_v8[:, sv0 + 1:sv0 + svn:2, 0:64],
                                              vo[:, :, 0:64])
                    vstate["done"] += svn

                for j in range(NCH):
                    qs = slice(j * QC, (j + 1) * QC)
                    sb_exp = big.tile([128, NKT, QC], FP8 if KF8 else BF16, tag="expS")
                    psP = psP_pool.tile([65, QC], F32, tag="pv")

                    state = {"pv": 0, "exp": 0}

                    def pv_upto(limit):
                        while True:
                            s = state["pv"]
                            if s >= NKT:
                                break
                            if KF8 and s % 2 == 0 and s + 1 < NKT:
                                if s + 2 > limit:
                                    break
                                nc.tensor.matmul(psP[:], sb_v8[:, s:s + 2, 0:65],
                                                 sb_exp[:, s:s + 2, :],
                                                 start=(s == 0), stop=False,
                                                 perf_mode=mybir.MatmulPerfMode.DoubleRow)
                                state["pv"] += 2
                            else:
                                if s + 1 > limit:
                                    break
                                nc.tensor.matmul(psP[:], sb_v8[:, s, 0:65],
                                                 sb_exp[:, s, :],
                                                 start=(s == 0), stop=(s == NKT - 1))
                                state["pv"] += 1

                    gtiles = {}
                    filled = [0] * len(groups)
                    if PACK:
                        steps = [[2 * p, 2 * p + 1] if p < 23 else [46] for p in range(24)]
                    else:
                        steps = [[s] for s in range(NKT)]
                    for slots in steps:
                        for s in slots:
                            g = s // 3
                            if g not in gtiles:
                                gtiles[g] = psS_pool.tile([128, EXPG * QC], F32, tag="s",
                                                          name=f"psS_{j}_{g}")
                        for hi, s in enumerate(slots):
                            lo = 64 * hi
                            tok = tok_of_slot(s) if PACK else s
                            col = (tok % 24) if PACK else tok
                            nc.tensor.matmul(
                                gtiles[s // 3][:, (s % 3) * QC:(s % 3 + 1) * QC],
                                sb_kt2[lo:lo + 64, col * 128:(col + 1) * 128],
                                sb_qtd[lo:lo + 64, qs], start=True, stop=True,
                                tile_position=(lo, 0) if PACK else None)
                        for s in slots:
                            g = s // 3
                            filled[g] += 1
                            if filled[g] == groups[g][1]:
                                gn = groups[g][1]
                                nc.scalar.activation(
                                    sb_exp[:, 3 * g:3 * g + gn, :],
                                    gtiles[g][:, :gn * QC], AF.Exp)
                                del gtiles[g]
                                state["exp"] += 1
                                emit_vbatch()
                                if state["exp"] >= 2:
                                    pv_upto(min(3 * (state["exp"] - 1),
                                                vstate["done"]))
                    pv_upto(NKT)

                    # softmax denominator + residual
                    if int(os.environ.get("KFASTRECIP", "0")):
                        nc.vector.reciprocal_approx_fast(sb_dinv[:, qs], psP[64:65, :])
                    else:
                        nc.vector.reciprocal(sb_dinv[:, qs], psP[64:65, :])
                    psR = psB_pool.tile([128, QC], F32, tag="bc")
                    nc.tensor.matmul(psR[:], sb_o128[:], sb_dinv[:, qs], start=True, stop=True)
                    sb_rb = work.tile([64, QC], F32, tag="rb")
                    nc.vector.tensor_copy(sb_rb[:], psR[0:64, :])
                    sb_a0 = work.tile([64, QC], F32, tag="a0")
                    nc.vector.tensor_mul(sb_a0[:], psP[0:64, :], sb_rb[:])
                    nc.vector.scalar_tensor_tensor(sb_att[:, qs], sb_a0[:], sb_bv[:],
                                                   sb_rawqt[:, qs],
                                                   op0=ALU.add, op1=ALU.add)
                    nc.vector.tensor_copy(sb_attB[0:64, qs], sb_att[:, qs])

                    # LN2 stats: mean row into attB[64], variance row into sb_vr
                    psM = psB_pool.tile([128, QC], F32, tag="bc")
                    nc.tensor.matmul(psM[0:1, :], sb_oc64[:], sb_attB[0:64, qs],
                                     start=True, stop=True)
                    nc.vector.tensor_copy(sb_attB[64:65, qs], psM[0:1, :])
                    sb_sq = work.tile([64, QC], BF16, tag="sq")
                    nc.vector.tensor_mul(sb_sq[:], sb_att[:, qs], sb_att[:, qs])
                    psM2 = psB_pool.tile([128, QC], F32, tag="bc")
                    nc.tensor.matmul(psM2[0:1, :], sb_oc64[:], sb_sq[:], start=True, stop=True)
                    sb_m2 = work.tile([1, QC], F32, tag="m2")
                    nc.vector.tensor_mul(sb_m2[:], sb_attB[64:65, qs], sb_attB[64:65, qs])
                    nc.vector.tensor_sub(sb_vr[:, qs], psM2[0:1, :], sb_m2[:])

            # ---------------- Phase C: FFN ----------------
            with tc.tile_pool(name="psC", bufs=2, space="PSUM") as psC:
                psHs = []
                for j in range(NCH):
                    qs = slice(j * QC, (j + 1) * QC)
                    psH = psC.tile([64, QC], F32, tag="h", bufs=3,
                                   name=f"psH_{j}")
                    nc.tensor.matmul(psH[:], sb_w1aug[:], sb_attB[:, qs],
                                     start=True, stop=True)
                    psHs.append(psH)
                sb_lv = big.tile([1, PQ], F32)
                nc.scalar.activation(sb_lv[:], sb_vr[:], AF.Ln, bias=1e-5)
                nc.scalar.activation(sb_rs[:], sb_lv[:], AF.Exp, scale=-0.5)
                nc.vector.tensor_copy(sb_rsb[:], sb_rs[:])
                for j in range(NCH):
                    qs = slice(j * QC, (j + 1) * QC)
                    psH = psHs[j]
                    psR2 = psC.tile([64, QC], F32, tag="rs")
                    nc.tensor.matmul(psR2[:], sb_o64b[:], sb_rsb[:, qs], start=True, stop=True)
                    sb_rsB = work.tile([64, QC], F32, tag="rsB")
                    nc.vector.tensor_copy(sb_rsB[:], psR2[:])
                    sb_hp = work.tile([64, QC], F32, tag="hp")
                    nc.vector.tensor_mul(sb_hp[:], psH[:], sb_rsB[:])
                    sb_gh = work.tile([64, QC], BF16, tag="gh")
                    nc.scalar.activation(sb_gh[:], sb_hp[:], AF.Gelu, bias=sb_b1f[:])
                    psU = psC.tile([64, QC], F32, tag="u")
                    nc.tensor.matmul(psU[:], sb_w2[:], sb_gh[:], start=True, stop=True)
                    sb_out = om.tile([64, QC], F32, tag="o")
                    nc.vector.scalar_tensor_tensor(sb_out[:], psU[:], sb_b2[:],
                                                   sb_att[:, qs], op0=ALU.add, op1=ALU.add)
                    nc.sync.dma_start(updt[:, qs], sb_out[:])
    nc.compile()
    return nc


_CACHE = {}


def _get_program():
    if "nc" not in _CACHE:
        _CACHE["nc"] = _build_program()
    return _CACHE["nc"]


def _host_prep(pillar_features, voxel_coords, Wq, bq, Wk, bk, Wv, bv,
               W1, b1, W2, b2, g1, be1, g2, be2):
    """Returns (in_maps, idx). All heavy parameterized compute stays on device."""
    pf = np.asarray(pillar_features, np.float32).reshape(B, P, C)
    vc = np.asarray(voxel_coords).astype(np.int64)
    idx = (vc[:, 1] + vc[:, 2] * NX + vc[:, 3]).reshape(B, P)

    theta = 10000.0 ** (-np.arange(HALF, dtype=np.float64) / HALF)
    h = np.linspace(-1.0, 1.0, NY)
    w = np.linspace(-1.0, 1.0, NX)
    hrow = h[idx // NX]
    wcol = w[idx % NX]
    A = np.cos(hrow[..., None] * theta) * np.cos(wcol[..., None] * theta)
    Bt = np.sin(hrow[..., None] * theta) * np.sin(wcol[..., None] * theta)
    x1 = pf[..., :HALF].astype(np.float64)
    x2 = pf[..., HALF:].astype(np.float64)
    raw = np.concatenate([x1 + x1 * A - x2 * Bt, x2 + x1 * Bt + x2 * A], axis=-1)

    g1 = np.asarray(g1, np.float64)
    be1 = np.asarray(be1, np.float64)
    m = raw.mean(-1, keepdims=True)
    v = ((raw - m) ** 2).mean(-1, keepdims=True)
    n1 = (raw - m) / np.sqrt(v + 1e-5) * g1 + be1

    raw32 = raw.astype(np.float32)
    n1b = n1.astype(NP_BF16)

    W1 = np.asarray(W1, np.float32)
    g2 = np.asarray(g2, np.float32)
    be2 = np.asarray(be2, np.float32)
    W1g = g2[:, None] * W1
    w1aug = np.zeros((65, 64), np.float32)
    w1aug[:64] = W1g
    w1aug[64] = -W1g.sum(axis=0)
    b1f = (be2 @ W1 + np.asarray(b1, np.float32)).reshape(64, 1)

    def dup(a):
        return np.vstack([a, a])

    weights = dict(
        wq=dup(np.asarray(Wq, np.float32).astype(NP_BF16)),
        bq=np.asarray(bq, np.float32).reshape(64, 1),
        wk=dup(np.asarray(Wk, np.float32).astype(NP_BF16)),
        bk=np.asarray(bk, np.float32).reshape(64, 1),
        wv=dup(np.asarray(Wv, np.float32).astype(NP_BF16)),
        bv=np.asarray(bv, np.float32).reshape(64, 1),
        w1aug=w1aug.astype(NP_BF16),
        b1f=b1f.astype(np.float32),
        w2=np.asarray(W2, np.float32).astype(NP_BF16),
        b2=np.asarray(b2, np.float32).reshape(64, 1).astype(np.float32),
    )

    n1t_s = []
    for b in range(B):
        t = np.zeros((64, KP), NP_BF16)
        t[:, :P] = n1b[b].T
        n1t_s.append(dup(t))

    in_maps = []
    for core in range(NCORES):
        b, s = divmod(core, QSH)
        q0 = s * PQR
        qn1t = np.zeros((64, PQ), NP_BF16)
        qn1t[:, :PQR] = n1b[b, q0:q0 + PQR].T
        rawqt = np.zeros((64, PQ), np.float32)
        rawqt[:, :PQR] = raw32[b, q0:q0 + PQR].T
        in_maps.append(dict(n1t=n1t_s[b], qn1t=dup(qn1t), rawqt=rawqt, **weights))
    return in_maps, idx


def _assemble(results, idx):
    out = np.zeros((B, C, HW), np.float32)
    for core in range(NCORES):
        b, s = divmod(core, QSH)
        q0 = s * PQR
        updt = np.asarray(results[core]["updt"], np.float32)
        out[b][:, idx[b, q0:q0 + PQR]] = updt[:, :PQR]
    return out.reshape(B, C, NY, NX)


def _run_device(in_maps, **kwargs):
    from concourse.bass_utils import run_bass_kernel_spmd
    nc = _get_program()
    return run_bass_kernel_spmd(nc, in_maps, list(range(NCORES)), **kwargs)


def kernel(**inputs):
    in_maps, idx = _host_prep(**inputs)
    res = _run_device(in_maps)
    return _assemble(res.results, idx)


if __name__ == "__main__":
    nc = _get_program()
    print("program built OK")


# revision 38
# speedup vs baseline: 1.0989x; 1.0850x over previous
"""Trainium2 Bass kernel: PointPillarScatter + dense self-attention block.

Math identities exploited (host side, exact):
  - The grid indices are unique per sample, so scatter->RoPE->gather equals
    applying RoPE per token at its grid position, and the final output grid
    is exactly zero away from the pillar columns (rot == 0 where grid == 0).
  - LN1 is folded on host; LN2's g2/be2 and mean-subtraction are folded into
    an augmented W1 (extra contraction row carrying the per-token mean).

Device kernel (SPMD over 8 cores = 2 samples x 4 query shards):
  QKV projections, S^T = K @ Q^T scores (keys on partitions), exp on ScalarE,
  PV with an extra all-ones V column producing the softmax denominators inside
  the same PSUM accumulation, the un-normalize + residual, LN2 stats via
  ones-matmuls, and the GELU FFN. All matmuls bf16 with fp32 PSUM; residual
  paths fp32.
"""

import os
import sys

import numpy as np

sys.path.insert(0, "/opt/trn_rl_repo")

import concourse.bass as bass  # noqa: E402
import concourse.bacc as bacc  # noqa: E402
import concourse.tile as tile  # noqa: E402
from concourse import mybir  # noqa: E402

dt = mybir.dt
F32 = dt.float32
BF16 = dt.bfloat16
FP8 = dt.float8e4
AF = mybir.ActivationFunctionType
ALU = mybir.AluOpType
NP_BF16 = dt.np(BF16)

B, P, C, HALF = 2, 6000, 64, 32
NX, NY = 432, 496
HW = NX * NY
NCORES = 8
QSH = 4             # query shards per sample
PQR = P // QSH      # 1500 real queries per core
QC = 512            # query chunk (one PSUM bank of fp32)
NCH = 3             # chunks per core
PQ = QC * NCH       # 1536 padded queries per core
NKT = 47            # key tiles of 128
KP = NKT * 128      # 6016 padded keys
VS = 66             # PSUM stride of one V tile (65 cols used, 8B aligned)
VS8 = 80            # SBUF stride of one V tile in the PV weight buffer (16B aligned)
EXPG = 3            # PSUM banks (key tiles) per exp group
PACK = int(os.environ.get("KPACK", "1"))  # row-packed paired S^T matmuls
KF8 = int(os.environ.get("KF8", "1"))     # fp8 DoubleRow PV matmuls
KWARM = int(os.environ.get("KWARM", "0"))  # PE warmup matmuls


def tok_of_slot(s):
    """Consumption slot -> token tile. With PACK, even slots are the A half
    (token tiles 0..23, kt2 partitions 0:64), odd the B half (24..46)."""
    if not PACK:
        return s
    return (s // 2) if s % 2 == 0 else 24 + (s // 2)


def _build_program():
    nc = bacc.Bacc("TRN2", target_bir_lowering=False, debug=False)

    def prm(name, shape, d, out=False):
        return nc.declare_dram_parameter(name, list(shape), d, isOutput=out)

    n1t = prm("n1t", (128, KP), BF16)       # LN1'd feats^T, duplicated in both halves
    qn1t = prm("qn1t", (128, PQ), BF16)     # query-side feats^T, duplicated halves
    rawqt = prm("rawqt", (64, PQ), F32)     # raw feats^T (residual) for queries
    wq = prm("wq", (128, 64), BF16)         # weights duplicated in both halves
    bq = prm("bq", (64, 1), F32)
    wk = prm("wk", (128, 64), BF16)
    bk = prm("bk", (64, 1), F32)
    wv = prm("wv", (128, 64), BF16)
    bv = prm("bv", (64, 1), F32)            # folded in after softmax (sum w = 1)
    w1aug = prm("w1aug", (65, 64), BF16)    # [g2*W1 ; -colsum(g2*W1)]
    b1f = prm("b1f", (64, 1), F32)          # be2 @ W1 + b1
    w2 = prm("w2", (64, 64), BF16)
    b2 = prm("b2", (64, 1), F32)
    updt = prm("updt", (64, PQ), F32, True)  # output: updated feats^T for queries

    groups = [(k0, min(EXPG, NKT - k0)) for k0 in range(0, NKT, EXPG)]

    with tile.TileContext(nc) as tc:
        with tc.tile_pool(name="const", bufs=1) as const, \
             tc.tile_pool(name="big", bufs=1) as big, \
             tc.tile_pool(name="work", bufs=3) as work, \
             tc.tile_pool(name="om", bufs=2) as om:

            # ---------------- persistent SBUF ----------------
            def ld(p, shape, d, name):
                t = const.tile(list(shape), d, tag=name)
                nc.sync.dma_start(t[:], p[:])
                return t

            sb_wq = ld(wq, (128, 64), BF16, "wq")
            sb_bq = ld(bq, (64, 1), F32, "bq")
            sb_wk = ld(wk, (128, 64), BF16, "wk")
            sb_bk = ld(bk, (64, 1), F32, "bk")
            sb_wv = ld(wv, (128, 64), BF16, "wv")
            sb_bv = ld(bv, (64, 1), F32, "bv")
            sb_w1aug = ld(w1aug, (65, 64), BF16, "w1aug")
            sb_b1f = ld(b1f, (64, 1), F32, "b1f")
            sb_w2 = ld(w2, (64, 64), BF16, "w2")
            sb_b2 = ld(b2, (64, 1), F32, "b2")

            sb_qn1t = big.tile([128, PQ], BF16)
            nc.sync.dma_start(sb_qn1t[:], qn1t[:])
            sb_n1t = big.tile([128, KP], BF16)
            for c0 in range(0, KP, QC):
                cn = min(QC, KP - c0)
                nc.sync.dma_start(sb_n1t[:, c0:c0 + cn], n1t[:, c0:c0 + cn])
            sb_rawqt = big.tile([64, PQ], F32)
            nc.sync.dma_start(sb_rawqt[:], rawqt[:])

            sb_zero = const.tile([128, 1], F32)
            nc.vector.memset(sb_zero[:], 0.0)
            nc.const_aps.aps[(F32, 0.0)] = sb_zero[:]
            sb_eps = const.tile([128, 1], F32)
            nc.vector.memset(sb_eps[:], 1e-5)
            nc.const_aps.aps[(F32, 1e-5)] = sb_eps[:]

            sb_o128 = const.tile([1, 128], F32)
            nc.vector.memset(sb_o128[:], 1.0)
            sb_o64b = const.tile([1, 64], BF16)
            nc.vector.memset(sb_o64b[:], 1.0)
            sb_oc64 = const.tile([64, 1], BF16)
            nc.vector.memset(sb_oc64[:], 1.0 / 64.0)

            # Q^T duplicated into both partition halves (rhs for row-packed S^T)
            sb_qtd = big.tile([128 if PACK else 64, PQ], BF16)
            # K^T split-half layout: [0:64] = token tiles 0..23, [64:128] = 24..46
            sb_kt2 = big.tile([128, 24 * 128] if PACK else [64, KP], BF16)
            sb_v8 = big.tile([128, 48, VS8], FP8 if KF8 else BF16)
            sb_att = big.tile([64, PQ], F32)     # att = attn@V/den + raw
            sb_attB = big.tile([65, PQ], BF16)   # bf16 att + mean row
            sb_vr = big.tile([1, PQ], F32)       # LN2 variance row
            sb_dinv = big.tile([1, PQ], F32)     # softmax 1/denominator
            sb_rs = big.tile([1, PQ], F32)       # LN2 1/sigma
            sb_rsb = big.tile([1, PQ], BF16)

            # ---------------- Phase A: Q, K, V ----------------
            with tc.tile_pool(name="psA", bufs=3, space="PSUM") as psA:
                if KWARM:
                    # spin the PE while input DMAs stream so HAM un-throttles
                    # (4096-cycle busy window) before the real matmuls start
                    sb_wu = const.tile([64, 512], BF16, tag="warm")
                    nc.vector.memset(sb_wu[:], 0.0)
                    psW = psA.tile([128, QC], F32, tag="qk", bufs=4)
                    for _ in range(KWARM):
                        nc.tensor.matmul(psW[:], sb_wu[:, 0:128], sb_wu[:],
                                         start=True, stop=True)
                state_a = []

                def proj_pair(jobs):
                    # jobs: list of (weights, half_lo, src_cols_n, dst, bias)
                    pss = []
                    for (w, lo, cols, n, src, dst, bias) in jobs:
                        ps = psA.tile([64, QC], F32, tag="qk", bufs=4,
                                      name=f"psQK_{len(state_a)}")
                        state_a.append(0)
                        nc.tensor.matmul(ps[:, :n], w[lo:lo + 64, :],
                                         src[lo:lo + 64, cols:cols + n],
                                         start=True, stop=True,
                                         tile_position=(lo, 0) if PACK else None)
                        pss.append((ps, n, dst, bias))
                    for (ps, n, dst, bias) in pss:
                        nc.vector.tensor_scalar_add(dst, ps[:, :n], bias)

                # Q: chunks 0..2 (pair first two across halves when packing)
                if PACK:
                    qjobs = [[(sb_wq, 0, 0, QC, sb_qn1t, None, sb_bq),
                              (sb_wq, 64, QC, QC, sb_qn1t, None, sb_bq)],
                             [(sb_wq, 0, 2 * QC, QC, sb_qn1t, None, sb_bq)]]
                else:
                    qjobs = [[(sb_wq, 0, j * QC, QC, sb_qn1t, None, sb_bq)]
                             for j in range(NCH)]
                for jobs in qjobs:
                    jj = [(w, lo, cols, n, src,
                           sb_qtd[0:64, cols:cols + n], bias)
                          for (w, lo, cols, n, src, _, bias) in jobs]
                    proj_pair(jj)
                if PACK:
                    for j in range(NCH):
                        nc.vector.tensor_copy(sb_qtd[64:128, j * QC:(j + 1) * QC],
                                              sb_qtd[0:64, j * QC:(j + 1) * QC])

                # K: 12 chunks; pair (j, j+6) across halves when packing
                if PACK:
                    for j0 in range(6):
                        nB = min(QC, KP - (j0 + 6) * QC)
                        jobs = [(sb_wk, 0, j0 * QC, QC, sb_n1t,
                                 sb_kt2[0:64, j0 * QC:(j0 + 1) * QC], sb_bk),
                                (sb_wk, 64, (j0 + 6) * QC, nB, sb_n1t,
                                 sb_kt2[64:128, j0 * QC:j0 * QC + nB], sb_bk)]
                        proj_pair(jobs)
                else:
                    nkc = (KP + QC - 1) // QC
                    for j in range(nkc):
                        n = min(QC, KP - j * QC)
                        jobs = [(sb_wk, 0, j * QC, n, sb_n1t,
                                 sb_kt2[0:64, j * QC:j * QC + n], sb_bk)]
                        proj_pair(jobs)
                # softmax-denominator ones column; pad keys (rows 112:128 of
                # the pad slot) zeroed via legal 32-aligned partition bases
                pad_slot = 2 * (46 - 24) + 1 if PACK else 46
                nc.vector.memset(sb_v8[:, :, 64:65], 1.0)
                nc.vector.memset(sb_v8[96:128, pad_slot, 64:65], 0.0)
                nc.vector.memset(sb_v8[96:112, pad_slot, 64:65], 1.0)

            # ---------------- Phase B: attention ----------------
            with tc.tile_pool(name="psS", bufs=2, space="PSUM") as psS_pool, \
                 tc.tile_pool(name="psP", bufs=1, space="PSUM") as psP_pool, \
                 tc.tile_pool(name="psB", bufs=1, space="PSUM") as psB_pool:
                vstate = {"done": 0}

                def emit_vbatch():
                    # V projections ride the psS-tag PSUM rotation during
                    # chunk 0 so the first exps are not blocked behind them.
                    sv0 = vstate["done"]
                    if sv0 >= NKT:
                        return
                    svn = min(6, NKT - sv0)
                    vt = psS_pool.tile([128, EXPG, QC], F32, tag="s",
                                       name=f"psV_{sv0}")
                    for t in range(svn):
                        s = sv0 + t
                        tok = tok_of_slot(s)
                        lo = 64 * (s % 2) if PACK else 0
                        h, i = t % 2, t // 2
                        nc.tensor.matmul(vt[:, h, i * VS:i * VS + 64],
                                         sb_n1t[lo:lo + 64, tok * 128:(tok + 1) * 128],
                                         sb_wv[lo:lo + 64, :], start=True, stop=True,
                                         tile_position=(lo, 0) if PACK else None)
                    ne, no = (svn + 1) // 2, svn // 2
                    ve = vt[:, 0, 0:ne * VS].rearrange("p (i v) -> p i v", v=VS)
                    nc.vector.tensor_copy(sb_v8[:, sv0:sv0 + svn:2, 0:64],
                                          ve[:, :, 0:64])
                    if no:
                        vo = vt[:, 1, 0:no * VS].rearrange("p (i v) -> p i v", v=VS)
                        nc.vector.tensor_copy(sb_v8[:, sv0 + 1:sv0 + svn:2, 0:64],
                                              vo[:, :, 0:64])
                    vstate["done"] += svn

                for j in range(NCH):
                    qs = slice(j * QC, (j + 1) * QC)
                    sb_exp = big.tile([128, NKT, QC], FP8 if KF8 else BF16, tag="expS", bufs=2)
                    psP = psP_pool.tile([65, QC], F32, tag="pv")

                    state = {"pv": 0, "exp": 0}

                    def pv_upto(limit):
                        while True:
                            s = state["pv"]
                            if s >= NKT:
                                break
                            if KF8 and s % 2 == 0 and s + 1 < NKT:
                                if s + 2 > limit:
                                    break
                                nc.tensor.matmul(psP[:], sb_v8[:, s:s + 2, 0:65],
                                                 sb_exp[:, s:s + 2, :],
                                                 start=(s == 0), stop=False,
                                                 perf_mode=mybir.MatmulPerfMode.DoubleRow)
                                state["pv"] += 2
                            else:
                                if s + 1 > limit:
                                    break
                                nc.tensor.matmul(psP[:], sb_v8[:, s, 0:65],
                                                 sb_exp[:, s, :],
                                                 start=(s == 0), stop=(s == NKT - 1))
                                state["pv"] += 1

                    gtiles = {}
                    filled = [0] * len(groups)
                    if PACK:
                        steps = [[2 * p, 2 * p + 1] if p < 23 else [46] for p in range(24)]
                    else:
                        steps = [[s] for s in range(NKT)]
                    for slots in steps:
                        for s in slots:
                            g = s // 3
                            if g not in gtiles:
                                gtiles[g] = psS_pool.tile([128, EXPG * QC], F32, tag="s",
                                                          name=f"psS_{j}_{g}")
                        for hi, s in enumerate(slots):
                            lo = 64 * hi
                            tok = tok_of_slot(s) if PACK else s
                            col = (tok % 24) if PACK else tok
                            nc.tensor.matmul(
                                gtiles[s // 3][:, (s % 3) * QC:(s % 3 + 1) * QC],
                                sb_kt2[lo:lo + 64, col * 128:(col + 1) * 128],
                                sb_qtd[lo:lo + 64, qs], start=True, stop=True,
                                tile_position=(lo, 0) if PACK else None)
                        for s in slots:
                            g = s // 3
                            filled[g] += 1
                            if filled[g] == groups[g][1]:
                                gn = groups[g][1]
                                nc.scalar.activation(
                                    sb_exp[:, 3 * g:3 * g + gn, :],
                                    gtiles[g][:, :gn * QC], AF.Exp)
                                del gtiles[g]
                                state["exp"] += 1
                                emit_vbatch()
                                if state["exp"] >= 2:
                                    pv_upto(min(3 * (state["exp"] - 1),
                                                vstate["done"]))
                    pv_upto(NKT)

                    # softmax denominator + residual. The reciprocal of the
                    # (1, 512) row is spread over 128 partitions via SBUF->SBUF
                    # DMA so the iterative-divide runs 4 elems/lane, not 512.
                    sb_dr = work.tile([1, QC], F32, tag="dr")
                    nc.vector.tensor_copy(sb_dr[:], psP[64:65, :])
                    sb_dq = work.tile([128, QC // 128], F32, tag="dq")
                    nc.sync.dma_start(
                        sb_dq[:], sb_dr[:].rearrange("o (p f) -> o p f", p=128))
                    sb_dq2 = work.tile([128, QC // 128], F32, tag="dq2")
                    nc.vector.reciprocal(sb_dq2[:], sb_dq[:])
                    nc.sync.dma_start(
                        sb_dinv[:, qs].rearrange("o (p f) -> o p f", p=128),
                        sb_dq2[:])
                    psR = psB_pool.tile([128, QC], F32, tag="bc")
                    nc.tensor.matmul(psR[:], sb_o128[:], sb_dinv[:, qs], start=True, stop=True)
                    sb_rb = work.tile([64, QC], F32, tag="rb")
                    nc.vector.tensor_copy(sb_rb[:], psR[0:64, :])
                    sb_a0 = work.tile([64, QC], F32, tag="a0")
                    nc.vector.tensor_mul(sb_a0[:], psP[0:64, :], sb_rb[:])
                    nc.vector.scalar_tensor_tensor(sb_att[:, qs], sb_a0[:], sb_bv[:],
                                                   sb_rawqt[:, qs],
                                                   op0=ALU.add, op1=ALU.add)
                    nc.vector.tensor_copy(sb_attB[0:64, qs], sb_att[:, qs])

                    # LN2 stats: mean row into attB[64], variance row into sb_vr
                    psM = psB_pool.tile([128, QC], F32, tag="bc")
                    nc.tensor.matmul(psM[0:1, :], sb_oc64[:], sb_attB[0:64, qs],
                                     start=True, stop=True)
                    nc.vector.tensor_copy(sb_attB[64:65, qs], psM[0:1, :])
                    sb_sq = work.tile([64, QC], BF16, tag="sq")
                    nc.vector.tensor_mul(sb_sq[:], sb_att[:, qs], sb_att[:, qs])
                    psM2 = psB_pool.tile([128, QC], F32, tag="bc")
                    nc.tensor.matmul(psM2[0:1, :], sb_oc64[:], sb_sq[:], start=True, stop=True)
                    sb_m2 = work.tile([1, QC], F32, tag="m2")
                    nc.vector.tensor_mul(sb_m2[:], sb_attB[64:65, qs], sb_attB[64:65, qs])
                    nc.vector.tensor_sub(sb_vr[:, qs], psM2[0:1, :], sb_m2[:])

            # ---------------- Phase C: FFN ----------------
            with tc.tile_pool(name="psC", bufs=2, space="PSUM") as psC:
                psHs = []
                for j in range(NCH):
                    qs = slice(j * QC, (j + 1) * QC)
                    psH = psC.tile([64, QC], F32, tag="h", bufs=3,
                                   name=f"psH_{j}")
                    nc.tensor.matmul(psH[:], sb_w1aug[:], sb_attB[:, qs],
                                     start=True, stop=True)
                    psHs.append(psH)
                sb_lv = big.tile([1, PQ], F32)
                nc.scalar.activation(sb_lv[:], sb_vr[:], AF.Ln, bias=1e-5)
                nc.scalar.activation(sb_rs[:], sb_lv[:], AF.Exp, scale=-0.5)
                nc.vector.tensor_copy(sb_rsb[:], sb_rs[:])
                for j in range(NCH):
                    qs = slice(j * QC, (j + 1) * QC)
                    psH = psHs[j]
                    psR2 = psC.tile([64, QC], F32, tag="rs")
                    nc.tensor.matmul(psR2[:], sb_o64b[:], sb_rsb[:, qs], start=True, stop=True)
                    sb_rsB = work.tile([64, QC], F32, tag="rsB")
                    nc.vector.tensor_copy(sb_rsB[:], psR2[:])
                    sb_hp = work.tile([64, QC], F32, tag="hp")
                    nc.vector.tensor_mul(sb_hp[:], psH[:], sb_rsB[:])
                    sb_gh = work.tile([64, QC], BF16, tag="gh")
                    nc.scalar.activation(sb_gh[:], sb_hp[:], AF.Gelu, bias=sb_b1f[:])
                    psU = psC.tile([64, QC], F32, tag="u")
                    nc.tensor.matmul(psU[:], sb_w2[:], sb_gh[:], start=True, stop=True)
                    sb_out = om.tile([64, QC], F32, tag="o")
                    nc.vector.scalar_tensor_tensor(sb_out[:], psU[:], sb_b2[:],
                                                   sb_att[:, qs], op0=ALU.add, op1=ALU.add)
                    nc.sync.dma_start(updt[:, qs], sb_out[:])
    nc.compile()
    return nc


_CACHE = {}


def _get_program():
    if "nc" not in _CACHE:
        _CACHE["nc"] = _build_program()
    return _CACHE["nc"]


def _host_prep(pillar_features, voxel_coords, Wq, bq, Wk, bk, Wv, bv,
               W1, b1, W2, b2, g1, be1, g2, be2):
    """Returns (in_maps, idx). All heavy parameterized compute stays on device."""
    pf = np.asarray(pillar_features, np.float32).reshape(B, P, C)
    vc = np.asarray(voxel_coords).astype(np.int64)
    idx = (vc[:, 1] + vc[:, 2] * NX + vc[:, 3]).reshape(B, P)

    theta = 10000.0 ** (-np.arange(HALF, dtype=np.float64) / HALF)
    h = np.linspace(-1.0, 1.0, NY)
    w = np.linspace(-1.0, 1.0, NX)
    hrow = h[idx // NX]
    wcol = w[idx % NX]
    A = np.cos(hrow[..., None] * theta) * np.cos(wcol[..., None] * theta)
    Bt = np.sin(hrow[..., None] * theta) * np.sin(wcol[..., None] * theta)
    x1 = pf[..., :HALF].astype(np.float64)
    x2 = pf[..., HALF:].astype(np.float64)
    raw = np.concatenate([x1 + x1 * A - x2 * Bt, x2 + x1 * Bt + x2 * A], axis=-1)

    g1 = np.asarray(g1, np.float64)
    be1 = np.asarray(be1, np.float64)
    m = raw.mean(-1, keepdims=True)
    v = ((raw - m) ** 2).mean(-1, keepdims=True)
    n1 = (raw - m) / np.sqrt(v + 1e-5) * g1 + be1

    raw32 = raw.astype(np.float32)
    n1b = n1.astype(NP_BF16)

    W1 = np.asarray(W1, np.float32)
    g2 = np.asarray(g2, np.float32)
    be2 = np.asarray(be2, np.float32)
    W1g = g2[:, None] * W1
    w1aug = np.zeros((65, 64), np.float32)
    w1aug[:64] = W1g
    w1aug[64] = -W1g.sum(axis=0)
    b1f = (be2 @ W1 + np.asarray(b1, np.float32)).reshape(64, 1)

    def dup(a):
        return np.vstack([a, a])

    weights = dict(
        wq=dup(np.asarray(Wq, np.float32).astype(NP_BF16)),
        bq=np.asarray(bq, np.float32).reshape(64, 1),
        wk=dup(np.asarray(Wk, np.float32).astype(NP_BF16)),
        bk=np.asarray(bk, np.float32).reshape(64, 1),
        wv=dup(np.asarray(Wv, np.float32).astype(NP_BF16)),
        bv=np.asarray(bv, np.float32).reshape(64, 1),
        w1aug=w1aug.astype(NP_BF16),
        b1f=b1f.astype(np.float32),
        w2=np.asarray(W2, np.float32).astype(NP_BF16),
        b2=np.asarray(b2, np.float32).reshape(64, 1).astype(np.float32),
    )

    n1t_s = []
    for b in range(B):
        t = np.zeros((64, KP), NP_BF16)
        t[:, :P] = n1b[b].T
        n1t_s.append(dup(t))

    in_maps = []
    for core in range(NCORES):
        b, s = divmod(core, QSH)
        q0 = s * PQR
        qn1t = np.zeros((64, PQ), NP_BF16)
        qn1t[:, :PQR] = n1b[b, q0:q0 + PQR].T
        rawqt = np.zeros((64, PQ), np.float32)
        rawqt[:, :PQR] = raw32[b, q0:q0 + PQR].T
        in_maps.append(dict(n1t=n1t_s[b], qn1t=dup(qn1t), rawqt=rawqt, **weights))
    return in_maps, idx


def _assemble(results, idx):
    out = np.zeros((B, C, HW), np.float32)
    for core in range(NCORES):
        b, s = divmod(core, QSH)
        q0 = s * PQR
        updt = np.asarray(results[core]["updt"], np.float32)
        out[b][:, idx[b, q0:q0 + PQR]] = updt[:, :PQR]
    return out.reshape(B, C, NY, NX)


def _run_device(in_maps, **kwargs):
    from concourse.bass_utils import run_bass_kernel_spmd
    nc = _get_program()
    return run_bass_kernel_spmd(nc, in_maps, list(range(NCORES)), **kwargs)


def kernel(**inputs):
    in_maps, idx = _host_prep(**inputs)
    res = _run_device(in_maps)
    return _assemble(res.results, idx)


if __name__ == "__main__":
    nc = _get_program()
    print("program built OK")


# revision 40
# speedup vs baseline: 1.1594x; 1.0551x over previous
"""Trainium2 Bass kernel: PointPillarScatter + dense self-attention block.

Math identities exploited (host side, exact):
  - The grid indices are unique per sample, so scatter->RoPE->gather equals
    applying RoPE per token at its grid position, and the final output grid
    is exactly zero away from the pillar columns (rot == 0 where grid == 0).
  - LN1 is folded on host; LN2's g2/be2 and mean-subtraction are folded into
    an augmented W1 (extra contraction row carrying the per-token mean).

Device kernel (SPMD over 8 cores = 2 samples x 4 query shards):
  QKV projections, S^T = K @ Q^T scores (keys on partitions), exp on ScalarE,
  PV with an extra all-ones V column producing the softmax denominators inside
  the same PSUM accumulation, the un-normalize + residual, LN2 stats via
  ones-matmuls, and the GELU FFN. All matmuls bf16 with fp32 PSUM; residual
  paths fp32.
"""

import os
import sys

import numpy as np

sys.path.insert(0, "/opt/trn_rl_repo")

import concourse.bass as bass  # noqa: E402
import concourse.bacc as bacc  # noqa: E402
import concourse.tile as tile  # noqa: E402
from concourse import mybir  # noqa: E402

dt = mybir.dt
F32 = dt.float32
BF16 = dt.bfloat16
FP8 = dt.float8e4
AF = mybir.ActivationFunctionType
ALU = mybir.AluOpType
NP_BF16 = dt.np(BF16)

B, P, C, HALF = 2, 6000, 64, 32
NX, NY = 432, 496
HW = NX * NY
NCORES = 8
QSH = 4             # query shards per sample
PQR = P // QSH      # 1500 real queries per core
QC = 512            # query chunk (one PSUM bank of fp32)
NCH = 3             # chunks per core
PQ = QC * NCH       # 1536 padded queries per core
NKT = 47            # key tiles of 128
KP = NKT * 128      # 6016 padded keys
VS = 66             # PSUM stride of one V tile (65 cols used, 8B aligned)
VS8 = 80            # SBUF stride of one V tile in the PV weight buffer (16B aligned)
EXPG = 3            # PSUM banks (key tiles) per exp group
PACK = int(os.environ.get("KPACK", "1"))  # row-packed paired S^T matmuls
KF8 = int(os.environ.get("KF8", "1"))     # fp8 DoubleRow PV matmuls
KWARM = int(os.environ.get("KWARM", "0"))  # PE warmup matmuls


def tok_of_slot(s):
    """Consumption slot -> token tile. With PACK, even slots are the A half
    (token tiles 0..23, kt2 partitions 0:64), odd the B half (24..46)."""
    if not PACK:
        return s
    return (s // 2) if s % 2 == 0 else 24 + (s // 2)


def _build_program():
    nc = bacc.Bacc("TRN2", target_bir_lowering=False, debug=False)

    def prm(name, shape, d, out=False):
        return nc.declare_dram_parameter(name, list(shape), d, isOutput=out)

    n1t = prm("n1t", (128, KP), BF16)       # LN1'd feats^T, duplicated in both halves
    qn1t = prm("qn1t", (128, PQ), BF16)     # query-side feats^T, duplicated halves
    rawqt = prm("rawqt", (64, PQ), F32)     # raw feats^T (residual) for queries
    # all bf16 weights in one blob: [wq | wk | wv (dup halves) | w1aug | w2]
    wblob = prm("wblob", (128, 320), BF16)
    # all fp32 per-channel vectors in one blob: [bq bk bv b1f b2 pad...]
    fblob = prm("fblob", (64, 8), F32)
    updt = prm("updt", (64, PQ), F32, True)  # output: updated feats^T for queries

    groups = [(k0, min(EXPG, NKT - k0)) for k0 in range(0, NKT, EXPG)]

    with tile.TileContext(nc) as tc:
        with tc.tile_pool(name="const", bufs=1) as const, \
             tc.tile_pool(name="big", bufs=1) as big, \
             tc.tile_pool(name="work", bufs=3) as work, \
             tc.tile_pool(name="om", bufs=2) as om:

            # ---------------- persistent SBUF ----------------
            sb_wblob = const.tile([128, 320], BF16, tag="wblob")
            nc.sync.dma_start(sb_wblob[:], wblob[:])
            sb_fblob = const.tile([64, 8], F32, tag="fblob")
            nc.sync.dma_start(sb_fblob[:], fblob[:])
            sb_wq = sb_wblob[:, 0:64]
            sb_wk = sb_wblob[:, 64:128]
            sb_wv = sb_wblob[:, 128:192]
            sb_w1aug = sb_wblob[0:65, 192:256]
            sb_w2 = sb_wblob[0:64, 256:320]
            sb_bq = sb_fblob[:, 0:1]
            sb_bk = sb_fblob[:, 1:2]
            sb_bv = sb_fblob[:, 2:3]
            sb_b1f = sb_fblob[:, 3:4]
            sb_b2 = sb_fblob[:, 4:5]

            sb_qn1t = big.tile([128, PQ], BF16)
            nc.sync.dma_start(sb_qn1t[:], qn1t[:])
            sb_n1t = big.tile([128, KP], BF16)
            for j0 in range(6):
                for c in (j0, j0 + 6):
                    c0 = c * QC
                    cn = min(QC, KP - c0)
                    nc.sync.dma_start(sb_n1t[:, c0:c0 + cn], n1t[:, c0:c0 + cn])
            sb_rawqt = big.tile([64, PQ], F32)
            nc.sync.dma_start(sb_rawqt[:], rawqt[:])

            sb_zero = const.tile([128, 1], F32)
            nc.vector.memset(sb_zero[:], 0.0)
            nc.const_aps.aps[(F32, 0.0)] = sb_zero[:]
            sb_eps = const.tile([128, 1], F32)
            nc.vector.memset(sb_eps[:], 1e-5)
            nc.const_aps.aps[(F32, 1e-5)] = sb_eps[:]

            sb_o128 = const.tile([1, 128], F32)
            nc.vector.memset(sb_o128[:], 1.0)
            sb_o64b = const.tile([1, 64], BF16)
            nc.vector.memset(sb_o64b[:], 1.0)
            sb_oc64 = const.tile([64, 1], BF16)
            nc.vector.memset(sb_oc64[:], 1.0 / 64.0)

            # Q^T duplicated into both partition halves (rhs for row-packed S^T)
            sb_qtd = big.tile([128 if PACK else 64, PQ], BF16)
            # K^T split-half layout: [0:64] = token tiles 0..23, [64:128] = 24..46
            sb_kt2 = big.tile([128, 24 * 128] if PACK else [64, KP], BF16)
            sb_v8 = big.tile([128, 48, VS8], FP8 if KF8 else BF16)
            # softmax-denominator ones column; pad keys (rows 112:128 of the
            # pad slot) zeroed. On GpSimd to stay off the DVE clock chain.
            pad_slot = 2 * (46 - 24) + 1 if PACK else 46
            nc.gpsimd.memset(sb_v8[:, :, 64:65], 1.0)
            nc.gpsimd.memset(sb_v8[96:128, pad_slot, 64:65], 0.0)
            nc.gpsimd.memset(sb_v8[96:112, pad_slot, 64:65], 1.0)
            sb_att = big.tile([64, PQ], F32)     # att = attn@V/den + raw
            sb_attB = big.tile([65, PQ], BF16)   # bf16 att + mean row
            sb_vr = big.tile([1, PQ], F32)       # LN2 variance row
            sb_dinv = big.tile([1, PQ], F32)     # softmax 1/denominator
            sb_rs = big.tile([1, PQ], F32)       # LN2 1/sigma
            sb_rsb = big.tile([1, PQ], BF16)

            # ---------------- Phase A: Q, K, V ----------------
            with tc.tile_pool(name="psA", bufs=3, space="PSUM") as psA:
                if KWARM:
                    # spin the PE while input DMAs stream so HAM un-throttles
                    # (4096-cycle busy window) before the real matmuls start
                    sb_wu = const.tile([64, 512], BF16, tag="warm")
                    nc.vector.memset(sb_wu[:], 0.0)
                    psW = psA.tile([128, QC], F32, tag="qk", bufs=4)
                    for _ in range(KWARM):
                        nc.tensor.matmul(psW[:], sb_wu[:, 0:128], sb_wu[:],
                                         start=True, stop=True)
                state_a = []

                def proj_pair(jobs):
                    # jobs: list of (weights, half_lo, src_cols_n, dst, bias)
                    pss = []
                    for (w, lo, cols, n, src, dst, bias) in jobs:
                        ps = psA.tile([64, QC], F32, tag="qk", bufs=4,
                                      name=f"psQK_{len(state_a)}")
                        state_a.append(0)
                        nc.tensor.matmul(ps[:, :n], w[lo:lo + 64, :],
                                         src[lo:lo + 64, cols:cols + n],
                                         start=True, stop=True,
                                         tile_position=(lo, 0) if PACK else None)
                        pss.append((ps, n, dst, bias))
                    for (ps, n, dst, bias) in pss:
                        nc.vector.tensor_scalar_add(dst, ps[:, :n], bias)

                # Q: chunks 0..2 (pair first two across halves when packing)
                if PACK:
                    qjobs = [[(sb_wq, 0, 0, QC, sb_qn1t, None, sb_bq),
                              (sb_wq, 64, QC, QC, sb_qn1t, None, sb_bq)],
                             [(sb_wq, 0, 2 * QC, QC, sb_qn1t, None, sb_bq)]]
                else:
                    qjobs = [[(sb_wq, 0, j * QC, QC, sb_qn1t, None, sb_bq)]
                             for j in range(NCH)]
                for jobs in qjobs:
                    jj = [(w, lo, cols, n, src,
                           sb_qtd[0:64, cols:cols + n], bias)
                          for (w, lo, cols, n, src, _, bias) in jobs]
                    proj_pair(jj)
                if PACK:
                    for j in range(NCH):
                        nc.vector.tensor_copy(sb_qtd[64:128, j * QC:(j + 1) * QC],
                                              sb_qtd[0:64, j * QC:(j + 1) * QC])

                # K: 12 chunks; pair (j, j+6) across halves when packing
                if PACK:
                    for j0 in range(6):
                        nB = min(QC, KP - (j0 + 6) * QC)
                        jobs = [(sb_wk, 0, j0 * QC, QC, sb_n1t,
                                 sb_kt2[0:64, j0 * QC:(j0 + 1) * QC], sb_bk),
                                (sb_wk, 64, (j0 + 6) * QC, nB, sb_n1t,
                                 sb_kt2[64:128, j0 * QC:j0 * QC + nB], sb_bk)]
                        proj_pair(jobs)
                else:
                    nkc = (KP + QC - 1) // QC
                    for j in range(nkc):
                        n = min(QC, KP - j * QC)
                        jobs = [(sb_wk, 0, j * QC, n, sb_n1t,
                                 sb_kt2[0:64, j * QC:j * QC + n], sb_bk)]
                        proj_pair(jobs)

            # ---------------- Phase B: attention ----------------
            with tc.tile_pool(name="psS", bufs=2, space="PSUM") as psS_pool, \
                 tc.tile_pool(name="psP", bufs=1, space="PSUM") as psP_pool, \
                 tc.tile_pool(name="psB", bufs=1, space="PSUM") as psB_pool:
                vstate = {"done": 0}

                def emit_vbatch():
                    # V projections ride the psS-tag PSUM rotation during
                    # chunk 0 so the first exps are not blocked behind them.
                    sv0 = vstate["done"]
                    if sv0 >= NKT:
                        return
                    svn = min(6, NKT - sv0)
                    vt = psS_pool.tile([128, EXPG, QC], F32, tag="s",
                                       name=f"psV_{sv0}")
                    for t in range(svn):
                        s = sv0 + t
                        tok = tok_of_slot(s)
                        lo = 64 * (s % 2) if PACK else 0
                        h, i = t % 2, t // 2
                        nc.tensor.matmul(vt[:, h, i * VS:i * VS + 64],
                                         sb_n1t[lo:lo + 64, tok * 128:(tok + 1) * 128],
                                         sb_wv[lo:lo + 64, :], start=True, stop=True,
                                         tile_position=(lo, 0) if PACK else None)
                    ne, no = (svn + 1) // 2, svn // 2
                    ve = vt[:, 0, 0:ne * VS].rearrange("p (i v) -> p i v", v=VS)
                    nc.vector.tensor_copy(sb_v8[:, sv0:sv0 + svn:2, 0:64],
                                          ve[:, :, 0:64])
                    if no:
                        vo = vt[:, 1, 0:no * VS].rearrange("p (i v) -> p i v", v=VS)
                        nc.vector.tensor_copy(sb_v8[:, sv0 + 1:sv0 + svn:2, 0:64],
                                              vo[:, :, 0:64])
                    vstate["done"] += svn

                for j in range(NCH):
                    qs = slice(j * QC, (j + 1) * QC)
                    sb_exp = big.tile([128, NKT, QC], FP8 if KF8 else BF16, tag="expS", bufs=2)
                    psP = psP_pool.tile([65, QC], F32, tag="pv")

                    state = {"pv": 0, "exp": 0}

                    def pv_upto(limit):
                        while True:
                            s = state["pv"]
                            if s >= NKT:
                                break
                            if KF8 and s % 2 == 0 and s + 1 < NKT:
                                if s + 2 > limit:
                                    break
                                nc.tensor.matmul(psP[:], sb_v8[:, s:s + 2, 0:65],
                                                 sb_exp[:, s:s + 2, :],
                                                 start=(s == 0), stop=False,
                                                 perf_mode=mybir.MatmulPerfMode.DoubleRow)
                                state["pv"] += 2
                            else:
                                if s + 1 > limit:
                                    break
                                nc.tensor.matmul(psP[:], sb_v8[:, s, 0:65],
                                                 sb_exp[:, s, :],
                                                 start=(s == 0), stop=(s == NKT - 1))
                                state["pv"] += 1

                    gtiles = {}
                    filled = [0] * len(groups)
                    if PACK:
                        steps = [[2 * p, 2 * p + 1] if p < 23 else [46] for p in range(24)]
                    else:
                        steps = [[s] for s in range(NKT)]
                    for slots in steps:
                        for s in slots:
                            g = s // 3
                            if g not in gtiles:
                                gtiles[g] = psS_pool.tile([128, EXPG * QC], F32, tag="s",
                                                          name=f"psS_{j}_{g}")
                        for hi, s in enumerate(slots):
                            lo = 64 * hi
                            tok = tok_of_slot(s) if PACK else s
                            col = (tok % 24) if PACK else tok
                            nc.tensor.matmul(
                                gtiles[s // 3][:, (s % 3) * QC:(s % 3 + 1) * QC],
                                sb_kt2[lo:lo + 64, col * 128:(col + 1) * 128],
                                sb_qtd[lo:lo + 64, qs], start=True, stop=True,
                                tile_position=(lo, 0) if PACK else None)
                        for s in slots:
                            g = s // 3
                            filled[g] += 1
                            if filled[g] == groups[g][1]:
                                gn = groups[g][1]
                                nc.scalar.activation(
                                    sb_exp[:, 3 * g:3 * g + gn, :],
                                    gtiles[g][:, :gn * QC], AF.Exp)
                                del gtiles[g]
                                state["exp"] += 1
                                emit_vbatch()
                                if state["exp"] >= 2:
                                    pv_upto(min(3 * (state["exp"] - 1),
                                                vstate["done"]))
                    pv_upto(NKT)

                    # softmax denominator + residual. The reciprocal of the
                    # (1, 512) row is spread over 128 partitions via SBUF->SBUF
                    # DMA so the iterative-divide runs 4 elems/lane, not 512.
                    sb_dr = work.tile([1, QC], F32, tag="dr")
                    nc.vector.tensor_copy(sb_dr[:], psP[64:65, :])
                    sb_dq = work.tile([128, QC // 128], F32, tag="dq")
                    nc.sync.dma_start(
                        sb_dq[:], sb_dr[:].rearrange("o (p f) -> o p f", p=128))
                    sb_dq2 = work.tile([128, QC // 128], F32, tag="dq2")
                    nc.vector.reciprocal(sb_dq2[:], sb_dq[:])
                    nc.sync.dma_start(
                        sb_dinv[:, qs].rearrange("o (p f) -> o p f", p=128),
                        sb_dq2[:])
                    psR = psB_pool.tile([128, QC], F32, tag="bc")
                    nc.tensor.matmul(psR[:], sb_o128[:], sb_dinv[:, qs], start=True, stop=True)
                    sb_rb = work.tile([64, QC], F32, tag="rb")
                    nc.vector.tensor_copy(sb_rb[:], psR[0:64, :])
                    sb_a0 = work.tile([64, QC], F32, tag="a0")
                    nc.vector.tensor_mul(sb_a0[:], psP[0:64, :], sb_rb[:])
                    nc.vector.scalar_tensor_tensor(sb_att[:, qs], sb_a0[:], sb_bv[:],
                                                   sb_rawqt[:, qs],
                                                   op0=ALU.add, op1=ALU.add)
                    nc.vector.tensor_copy(sb_attB[0:64, qs], sb_att[:, qs])

                    # LN2 stats: mean row into attB[64], variance row into sb_vr
                    psM = psB_pool.tile([128, QC], F32, tag="bc")
                    nc.tensor.matmul(psM[0:1, :], sb_oc64[:], sb_attB[0:64, qs],
                                     start=True, stop=True)
                    nc.vector.tensor_copy(sb_attB[64:65, qs], psM[0:1, :])
                    sb_sq = work.tile([64, QC], BF16, tag="sq")
                    nc.vector.tensor_mul(sb_sq[:], sb_att[:, qs], sb_att[:, qs])
                    psM2 = psB_pool.tile([128, QC], F32, tag="bc")
                    nc.tensor.matmul(psM2[0:1, :], sb_oc64[:], sb_sq[:], start=True, stop=True)
                    sb_m2 = work.tile([1, QC], F32, tag="m2")
                    nc.vector.tensor_mul(sb_m2[:], sb_attB[64:65, qs], sb_attB[64:65, qs])
                    nc.vector.tensor_sub(sb_vr[:, qs], psM2[0:1, :], sb_m2[:])

            # ---------------- Phase C: FFN ----------------
            with tc.tile_pool(name="psC", bufs=2, space="PSUM") as psC:
                psHs = []
                for j in range(NCH):
                    qs = slice(j * QC, (j + 1) * QC)
                    psH = psC.tile([64, QC], F32, tag="h", bufs=3,
                                   name=f"psH_{j}")
                    nc.tensor.matmul(psH[:], sb_w1aug[:], sb_attB[:, qs],
                                     start=True, stop=True)
                    psHs.append(psH)
                sb_lv = big.tile([1, PQ], F32)
                nc.scalar.activation(sb_lv[:], sb_vr[:], AF.Ln, bias=1e-5)
                nc.scalar.activation(sb_rs[:], sb_lv[:], AF.Exp, scale=-0.5)
                nc.vector.tensor_copy(sb_rsb[:], sb_rs[:])
                for j in range(NCH):
                    qs = slice(j * QC, (j + 1) * QC)
                    psH = psHs[j]
                    psR2 = psC.tile([64, QC], F32, tag="rs")
                    nc.tensor.matmul(psR2[:], sb_o64b[:], sb_rsb[:, qs], start=True, stop=True)
                    sb_rsB = work.tile([64, QC], F32, tag="rsB")
                    nc.vector.tensor_copy(sb_rsB[:], psR2[:])
                    sb_hp = work.tile([64, QC], F32, tag="hp")
                    nc.vector.tensor_mul(sb_hp[:], psH[:], sb_rsB[:])
                    sb_gh = work.tile([64, QC], BF16, tag="gh")
                    nc.scalar.activation(sb_gh[:], sb_hp[:], AF.Gelu, bias=sb_b1f[:])
                    psU = psC.tile([64, QC], F32, tag="u")
                    nc.tensor.matmul(psU[:], sb_w2[:], sb_gh[:], start=True, stop=True)
                    sb_out = om.tile([64, QC], F32, tag="o")
                    nc.vector.scalar_tensor_tensor(sb_out[:], psU[:], sb_b2[:],
                                                   sb_att[:, qs], op0=ALU.add, op1=ALU.add)
                    nc.sync.dma_start(updt[:, qs], sb_out[:])
    nc.compile()
    return nc


_CACHE = {}


def _get_program():
    if "nc" not in _CACHE:
        _CACHE["nc"] = _build_program()
    return _CACHE["nc"]


def _host_prep(pillar_features, voxel_coords, Wq, bq, Wk, bk, Wv, bv,
               W1, b1, W2, b2, g1, be1, g2, be2):
    """Returns (in_maps, idx). All heavy parameterized compute stays on device."""
    pf = np.asarray(pillar_features, np.float32).reshape(B, P, C)
    vc = np.asarray(voxel_coords).astype(np.int64)
    idx = (vc[:, 1] + vc[:, 2] * NX + vc[:, 3]).reshape(B, P)

    theta = 10000.0 ** (-np.arange(HALF, dtype=np.float64) / HALF)
    h = np.linspace(-1.0, 1.0, NY)
    w = np.linspace(-1.0, 1.0, NX)
    hrow = h[idx // NX]
    wcol = w[idx % NX]
    A = np.cos(hrow[..., None] * theta) * np.cos(wcol[..., None] * theta)
    Bt = np.sin(hrow[..., None] * theta) * np.sin(wcol[..., None] * theta)
    x1 = pf[..., :HALF].astype(np.float64)
    x2 = pf[..., HALF:].astype(np.float64)
    raw = np.concatenate([x1 + x1 * A - x2 * Bt, x2 + x1 * Bt + x2 * A], axis=-1)

    g1 = np.asarray(g1, np.float64)
    be1 = np.asarray(be1, np.float64)
    m = raw.mean(-1, keepdims=True)
    v = ((raw - m) ** 2).mean(-1, keepdims=True)
    n1 = (raw - m) / np.sqrt(v + 1e-5) * g1 + be1

    raw32 = raw.astype(np.float32)
    n1b = n1.astype(NP_BF16)

    W1 = np.asarray(W1, np.float32)
    g2 = np.asarray(g2, np.float32)
    be2 = np.asarray(be2, np.float32)
    W1g = g2[:, None] * W1
    w1aug = np.zeros((65, 64), np.float32)
    w1aug[:64] = W1g
    w1aug[64] = -W1g.sum(axis=0)
    b1f = (be2 @ W1 + np.asarray(b1, np.float32)).reshape(64, 1)

    def dup(a):
        return np.vstack([a, a])

    wb = np.zeros((128, 320), np.float32)
    wb[:, 0:64] = dup(np.asarray(Wq, np.float32))
    wb[:, 64:128] = dup(np.asarray(Wk, np.float32))
    wb[:, 128:192] = dup(np.asarray(Wv, np.float32))
    wb[0:65, 192:256] = w1aug
    wb[0:64, 256:320] = np.asarray(W2, np.float32)
    fb = np.zeros((64, 8), np.float32)
    fb[:, 0] = np.asarray(bq, np.float32)
    fb[:, 1] = np.asarray(bk, np.float32)
    fb[:, 2] = np.asarray(bv, np.float32)
    fb[:, 3] = b1f[:, 0]
    fb[:, 4] = np.asarray(b2, np.float32)
    weights = dict(wblob=wb.astype(NP_BF16), fblob=fb)

    n1t_s = []
    for b in range(B):
        t = np.zeros((64, KP), NP_BF16)
        t[:, :P] = n1b[b].T
        n1t_s.append(dup(t))

    in_maps = []
    for core in range(NCORES):
        b, s = divmod(core, QSH)
        q0 = s * PQR
        qn1t = np.zeros((64, PQ), NP_BF16)
        qn1t[:, :PQR] = n1b[b, q0:q0 + PQR].T
        rawqt = np.zeros((64, PQ), np.float32)
        rawqt[:, :PQR] = raw32[b, q0:q0 + PQR].T
        in_maps.append(dict(n1t=n1t_s[b], qn1t=dup(qn1t), rawqt=rawqt, **weights))
    return in_maps, idx


def _assemble(results, idx):
    out = np.zeros((B, C, HW), np.float32)
    for core in range(NCORES):
        b, s = divmod(core, QSH)
        q0 = s * PQR
        updt = np.asarray(results[core]["updt"], np.float32)
        out[b][:, idx[b, q0:q0 + PQR]] = updt[:, :PQR]
    return out.reshape(B, C, NY, NX)


def _run_device(in_maps, **kwargs):
    from concourse.bass_utils import run_bass_kernel_spmd
    nc = _get_program()
    return run_bass_kernel_spmd(nc, in_maps, list(range(NCORES)), **kwargs)


def kernel(**inputs):
    in_maps, idx = _host_prep(**inputs)
    res = _run_device(in_maps)
    return _assemble(res.results, idx)


if __name__ == "__main__":
    nc = _get_program()
    print("program built OK")
